# revision 1
# baseline (speedup 1.0000x reference)
"""GCN classifier kernel for Trainium2 (Bass/Tile), 8-core SPMD.

Math: for each GCN layer, relu(nd * (A^T (ns * h)) @ W + b)
  == relu(sum_e w_e * p[src_e] + b) aggregated per dst, where
  p = h @ W and w_e = ns[src_e] * nd[dst_e].
Layer 0 input h0 = in_deg (rank 1), so layer 1 collapses to
  h1 = relu(q1[:, None] * W0 + b0) with q1 host-precomputable from the
  graph alone.

Device pipeline per core (owns 6272 dst nodes = 49 blocks of 128):
  L1: h1T = relu(W0^T q1 + b0) per block; p1 = h1 @ W1 -> slab1
  AllGather slab1 -> table1 (replicated 50176 x 128)
  L2: dma_gather msgs = table1[src]; aggT += msg_chunk @ S_chunk (PE);
      h2T = relu(aggT + b1); p2 = h2 @ W2 -> slab2; AllGather -> table2
  L3: gather; agg += S_chunk^T @ msg_chunk; h3 = relu(agg + b2)
      readout: r_t += S_g(t)^T @ h3 (4 global graph tiles); r_t *= 1/cnt
  AllReduce partial [512,128]; out = rg @ Wc + bc  -> [512, 10]

S chunks are one-hot(dst) * w_e matrices generated on-device by DVE
tensor_scalar(is_equal, mult) from host-prepared per-chunk columns.
dma_gather indices are int16, so the 50176-row table is addressed via two
overlapping base windows (rows 0..32767 and 17408..50175).
"""

import sys

sys.path.insert(0, "/opt/trn_rl_repo")

import numpy as np

import concourse.bass as bass
import concourse.mybir as mybir
import concourse.tile as tile
from concourse import bacc, bass_utils

P = 128
N_CORES = 8
N_NODES = 50000
N_EDGES = 800000
HID = 128
N_GRAPHS = 512
N_CLASSES = 10

NPC = 6272          # nodes per core (49 blocks of 128)
BLOCKS = NPC // P   # 49
NPAD = NPC * N_CORES  # 50176
HALF0 = 32768       # gather window 0: rows [0, 32768)
BASE1 = NPAD - 32768  # 17408; window 1: rows [17408, 50176)
GA = 8              # gather group size in chunks of 128 edges (1024 idxs = HW cap per dma_gather)
F32 = mybir.dt.float32
I16 = mybir.dt.int16
I32 = mybir.dt.int32


def _prep_graph(src, dst, graph_ids):
    """Host-side preprocessing: degrees, q1, per-core edge schedule."""
    src = np.asarray(src).astype(np.int64)
    dst = np.asarray(dst).astype(np.int64)
    graph_ids = np.asarray(graph_ids).astype(np.int64)

    in_deg = np.bincount(dst, minlength=N_NODES).astype(np.float32)
    out_deg = np.bincount(src, minlength=N_NODES).astype(np.float32)
    ns = np.maximum(out_deg, 1.0) ** -0.5
    nd = np.maximum(in_deg, 1.0) ** -0.5
    # layer-1 aggregate: q1 = nd * segsum_dst((in_deg*ns)[src])
    c0 = (in_deg * ns).astype(np.float64)
    t1 = np.bincount(dst, weights=c0[src], minlength=N_NODES)
    q1 = (nd.astype(np.float64) * t1).astype(np.float32)

    w_edge = (ns[src] * nd[dst]).astype(np.float32)

    # per-core, per-block, per-half chunk counts
    blk_of = dst % NPC  # placeholder; computed per core below
    counts = np.zeros((N_CORES, BLOCKS, 2), np.int64)
    per_core = []
    for c in range(N_CORES):
        base = c * NPC
        m = (dst >= base) & (dst < base + NPC)
        es, ed, ew = src[m], dst[m], w_edge[m]
        dloc = ed - base
        blk = dloc >> 7
        # edges with src in [BASE1, HALF0) fit either gather window; assign
        # them per block to minimize chunk padding (ceil waste)
        half = (es >= HALF0).astype(np.int64)
        over = (es >= BASE1) & (es < HALF0)
        for b in range(BLOCKS):
            mb = blk == b
            n_low = int(np.count_nonzero(mb & (es < BASE1)))
            n_over = int(np.count_nonzero(mb & over))
            n_high = int(np.count_nonzero(mb & (es >= HALF0)))
            cands = {0, n_over}
            k = (-n_low) % P
            while k <= n_over:
                cands.add(k)
                k += P
            best_x, best_cost = 0, 10**9
            for x in sorted(cands):
                cost = -(-(n_low + x) // P) + -(-(n_high + n_over - x) // P)
                if cost < best_cost:
                    best_cost, best_x = cost, x
            if best_x < n_over:
                idxs_over = np.nonzero(mb & over)[0]
                half[idxs_over[best_x:]] = 1
        order = np.lexsort((es, half, blk))
        es, dloc, ew, blk, half = (
            es[order], dloc[order], ew[order], blk[order], half[order])
        for b in range(BLOCKS):
            mb = blk == b
            counts[c, b, 0] = np.count_nonzero(mb & (half == 0))
            counts[c, b, 1] = np.count_nonzero(mb & (half == 1))
        per_core.append((es, dloc, ew, blk, half))

    K0 = np.maximum(1, np.ceil(counts[:, :, 0] / P).max(axis=0).astype(np.int64))
    K1 = np.ceil(counts[:, :, 1] / P).max(axis=0).astype(np.int64)
    KA = int(K0.sum())
    KB = int(K1.sum())

    core_arrays = []
    for c in range(N_CORES):
        es, dloc, ew, blk, half = per_core[c]
        base = c * NPC
        idxA = np.zeros(KA * P, np.int32)
        dvA = np.zeros(KA * P, np.float32)
        wA = np.zeros(KA * P, np.float32)
        idxB = np.zeros(KB * P, np.int32)
        dvB = np.zeros(KB * P, np.float32)
        wB = np.zeros(KB * P, np.float32)
        offA = 0
        offB = 0
        for b in range(BLOCKS):
            for h, (idxs, dvs, ws, K, off) in enumerate((
                (idxA, dvA, wA, int(K0[b]), offA),
                (idxB, dvB, wB, int(K1[b]), offB),
            )):
                m = (blk == b) & (half == h)
                n = int(np.count_nonzero(m))
                assert n <= K * P
                sl = slice(off, off + n)
                idxs[sl] = es[m] - (0 if h == 0 else BASE1)
                dvs[sl] = (dloc[m] - b * P).astype(np.float32)
                ws[sl] = ew[m]
                # padding stays idx=0, dstv=0, w=0 (contributes 0 via S)
            offA += int(K0[b]) * P
            offB += int(K1[b]) * P

        def idx_layout(v):
            # index i -> partition i%16 (replicated x8), column i//16
            r = v.astype(np.int16).reshape(-1, 16).T  # [16, L/16]
            return np.tile(r, (8, 1)).copy()  # [128, L/16]

        def col_layout(v):
            return np.ascontiguousarray(v.reshape(-1, P).T)  # [128, K]

        own = np.arange(base, base + NPC)
        q1row = np.zeros((1, NPC), np.float32)
        real = own < N_NODES
        q1row[0, real] = q1[own[real]]
        gph = np.full(NPC, -1.0, np.float32)
        gph[real] = graph_ids[own[real]].astype(np.float32)

        core_arrays.append(dict(
            idxA=idx_layout(idxA), idxB=idx_layout(idxB),
            dvA=col_layout(dvA), wA=col_layout(wA),
            dvB=col_layout(dvB), wB=col_layout(wB),
            q1row=q1row,
            gphv=np.ascontiguousarray(gph.reshape(BLOCKS, P).T),
        ))

    cnt = np.bincount(graph_ids, minlength=N_GRAPHS).astype(np.float32)
    invc = (1.0 / np.maximum(cnt, 1.0)).reshape(N_GRAPHS // P, P).T  # [128, 4]
    invc = np.ascontiguousarray(invc)

    sched = dict(K0=K0, K1=K1, KA=KA, KB=KB)
    return sched, core_arrays, invc


def build_nc(sched, reps=1, with_coll=True, with_gather=True,
             with_sgen=True, with_compute=True, msg_bufs=16, sgen_bufs=12,
             hbuf_bufs=6):
    """Build and compile the 8-core SPMD Bass program.

    reps>1 repeats the whole pipeline inside one NEFF (for timing via
    slope); with_coll/with_gather=False drop those phases (timing only —
    results are garbage)."""
    K0, K1, KA, KB = sched["K0"], sched["K1"], sched["KA"], sched["KB"]
    NGT = N_GRAPHS // P  # 4

    nc = bacc.Bacc("TRN2", target_bir_lowering=False, debug=False,
                   num_devices=N_CORES, num_swdge_queues=4)

    def inp(name, shape, dt=F32):
        return nc.dram_tensor(name, list(shape), dt, kind="ExternalInput").ap()

    d_idxA = inp("idxA", [P, KA * 8], I16)
    d_idxB = inp("idxB", [P, max(KB, 1) * 8], I16)
    d_dvA = inp("dvA", [P, KA])
    d_wA = inp("wA", [P, KA])
    d_dvB = inp("dvB", [P, max(KB, 1)])
    d_wB = inp("wB", [P, max(KB, 1)])
    d_q1 = inp("q1row", [1, NPC])
    d_gph = inp("gphv", [P, BLOCKS])
    d_invc = inp("invc", [P, NGT])
    d_W0 = inp("W0", [1, HID])
    d_W1 = inp("W1", [HID, HID])
    d_W2 = inp("W2", [HID, HID])
    d_Wc = inp("Wc", [HID, N_CLASSES])
    d_b0c = inp("b0c", [P, 1])
    d_b1c = inp("b1c", [P, 1])
    d_b2r = inp("b2r", [P, HID])
    d_bcr = inp("bcr", [P, N_CLASSES])

    out = nc.dram_tensor("out", [N_GRAPHS, N_CLASSES], F32,
                         kind="ExternalOutput").ap()

    slab1 = nc.dram_tensor("slab1", [NPC, HID], F32, kind="Internal").ap()
    slab2 = nc.dram_tensor("slab2", [NPC, HID], F32, kind="Internal").ap()
    table1 = nc.dram_tensor("table1", [NPAD, HID], F32, kind="Internal",
                            addr_space="Shared").ap()
    table2 = nc.dram_tensor("table2", [NPAD, HID], F32, kind="Internal",
                            addr_space="Shared").ap()
    partial = nc.dram_tensor("partial", [N_GRAPHS, HID], F32,
                             kind="Internal").ap()
    summed = nc.dram_tensor("summed", [N_GRAPHS, HID], F32, kind="Internal",
                            addr_space="Shared").ap()

    RG = [list(range(N_CORES))]

    # block -> chunk ranges in streams A and B
    offA = np.concatenate([[0], np.cumsum(K0)]).astype(int)
    offB = np.concatenate([[0], np.cumsum(K1)]).astype(int)

    with tile.TileContext(nc) as tc:
        with tc.tile_pool(name="const", bufs=1) as cp, \
             tc.tile_pool(name="msg", bufs=msg_bufs) as mp, \
             tc.tile_pool(name="sgen", bufs=sgen_bufs) as sp, \
             tc.tile_pool(name="hbuf", bufs=hbuf_bufs) as hp, \
             tc.tile_pool(name="agg_ps", bufs=2, space="PSUM") as agg_ps, \
             tc.tile_pool(name="p_ps", bufs=2, space="PSUM") as p_ps, \
             tc.tile_pool(name="r_ps", bufs=1, space="PSUM") as r_ps:

            def load_const(ap_in, shape, dt=F32):
                t = cp.tile(list(shape), dt, tag=ap_in.name)
                nc.sync.dma_start(t[:], ap_in[:])
                return t

            idxA = load_const(d_idxA, [P, KA * 8], I16)
            idxB = load_const(d_idxB, [P, max(KB, 1) * 8], I16)
            dvA = load_const(d_dvA, [P, KA])
            wA = load_const(d_wA, [P, KA])
            dvB = load_const(d_dvB, [P, max(KB, 1)])
            wB = load_const(d_wB, [P, max(KB, 1)])
            q1r = load_const(d_q1, [1, NPC])
            gph = load_const(d_gph, [P, BLOCKS])
            invc = load_const(d_invc, [P, NGT])
            W0 = load_const(d_W0, [1, HID])
            W1 = load_const(d_W1, [HID, HID])
            W2 = load_const(d_W2, [HID, HID])
            Wc = load_const(d_Wc, [HID, N_CLASSES])
            b0c = load_const(d_b0c, [P, 1])
            b1c = load_const(d_b1c, [P, 1])
            b2r = load_const(d_b2r, [P, HID])
            bcr = load_const(d_bcr, [P, N_CLASSES])

            iota_i = cp.tile([P, P], I32, tag="iota_i")
            nc.gpsimd.iota(iota_i[:], pattern=[[1, P]], base=0,
                           channel_multiplier=0)
            iota_f = cp.tile([P, P], F32, tag="iota_f")
            nc.vector.tensor_copy(iota_f[:], iota_i[:])
            iotg_i = cp.tile([P, N_GRAPHS], I32, tag="iotg_i")
            nc.gpsimd.iota(iotg_i[:], pattern=[[1, N_GRAPHS]], base=0,
                           channel_multiplier=0)
            iotg_f = cp.tile([P, N_GRAPHS], F32, tag="iotg_f")
            nc.vector.tensor_copy(iotg_f[:], iotg_i[:])

            ident = cp.tile([P, P], F32, tag="ident")
            from concourse.masks import make_identity
            make_identity(nc, ident[:])

            RELU = mybir.ActivationFunctionType.Relu

            # block -> chunk list over both streams
            def block_chunks(b):
                res = []
                for ca in range(offA[b], offA[b + 1]):
                    res.append(("A", ca))
                for cb in range(offB[b], offB[b + 1]):
                    res.append(("B", cb))
                return res

            for rep in range(reps):
                # ---------------- layer 1 ----------------
                for k in range(BLOCKS):
                    h1T_psum = agg_ps.tile([P, P], F32, tag="aggps")
                    nc.tensor.matmul(
                        out=h1T_psum[:], lhsT=W0[:],
                        rhs=q1r[:][:, k * P:(k + 1) * P],
                        start=True, stop=True)
                    h1T = hp.tile([P, P], F32, tag="hT")
                    nc.scalar.activation(out=h1T[:], in_=h1T_psum[:],
                                         func=RELU, bias=b0c[:], scale=1.0)
                    p_psum = p_ps.tile([P, P], F32, tag="pps")
                    nc.tensor.matmul(out=p_psum[:], lhsT=h1T[:], rhs=W1[:],
                                     start=True, stop=True)
                    p_sb = hp.tile([P, P], F32, tag="pout")
                    nc.vector.tensor_copy(p_sb[:], p_psum[:])
                    nc.sync.dma_start(slab1[k * P:(k + 1) * P, :], p_sb[:])

                if with_coll:
                    nc.gpsimd.collective_compute(
                        "AllGather", mybir.AluOpType.bypass,
                        replica_groups=RG,
                        ins=[slab1[:]], outs=[table1[:]])

                def emit_gathers(table_ap):
                    """Gather + S-gen per group, block-sorted across streams.

                    Returns chunk -> (msg tile, S8 tile, col). Pool executes
                    gathers in order, so groups are emitted in first-use
                    (block) order to avoid msg-slot deadlock.
                    """
                    chunk_src = {}
                    groups = []
                    blockA = np.searchsorted(offA[1:], np.arange(KA),
                                             side="right")
                    blockB = np.searchsorted(offB[1:], np.arange(max(KB, 1)),
                                             side="right")
                    for stream, K, idx_t, blk_of in (
                            ("A", KA, idxA, blockA), ("B", KB, idxB, blockB)):
                        base_ap = (table_ap[0:HALF0, :] if stream == "A"
                                   else table_ap[BASE1:NPAD, :])
                        g0 = 0
                        while g0 < K:
                            ln = min(GA, K - g0)
                            groups.append(
                                (int(blk_of[g0]), stream, g0, ln, base_ap,
                                 idx_t))
                            g0 += ln
                    groups.sort(key=lambda g: (g[0], g[1]))
                    for gi, (_fb, stream, g0, ln, base_ap, idx_t) in \
                            enumerate(groups):
                        mt = mp.tile([P, GA * P], F32, tag="msg")
                        out_ap = mt[:][:, :ln * P].rearrange(
                            "p (a b) -> p a b", b=P)
                        if with_gather:
                            nc.gpsimd.dma_gather(
                                out_ap=out_ap, in_ap=base_ap,
                                idxs_ap=idx_t[:][:, g0 * 8:(g0 + ln) * 8],
                                num_idxs=ln * P, num_idxs_reg=ln * P,
                                elem_size=HID, queue_num=gi % 4)
                        # S for the whole group: two wide DVE ops
                        dv, w = (dvA, wA) if stream == "A" else (dvB, wB)
                        S8 = sp.tile([P, GA * P], F32, tag="S8")
                        s_ap = S8[:][:, :ln * P].rearrange(
                            "p (a b) -> p a b", b=P)
                        if with_sgen:
                            io8 = iota_f[:].unsqueeze(1).broadcast_to(
                                [P, ln, P])
                            dv8 = dv[:][:, g0:g0 + ln].unsqueeze(2). \
                                broadcast_to([P, ln, P])
                            w8 = w[:][:, g0:g0 + ln].unsqueeze(2). \
                                broadcast_to([P, ln, P])
                            nc.vector.tensor_tensor(
                                out=s_ap, in0=io8, in1=dv8,
                                op=mybir.AluOpType.is_equal)
                            nc.vector.tensor_tensor(
                                out=s_ap, in0=s_ap, in1=w8,
                                op=mybir.AluOpType.mult)
                        for j in range(ln):
                            chunk_src[(stream, g0 + j)] = (mt, S8, j)
                    return chunk_src

                # ---------------- layer 2 ----------------
                chunk_src = emit_gathers(table1)
                for b in range(BLOCKS if with_compute else 0):
                    chunks = block_chunks(b)
                    aggT = agg_ps.tile([P, P], F32, tag="aggps")
                    for j, (stream, ci) in enumerate(chunks):
                        mt, S8, col = chunk_src[(stream, ci)]
                        nc.tensor.matmul(
                            out=aggT[:],
                            lhsT=mt[:][:, col * P:(col + 1) * P],
                            rhs=S8[:][:, col * P:(col + 1) * P],
                            start=(j == 0), stop=(j == len(chunks) - 1))
                    h2T = hp.tile([P, P], F32, tag="hT")
                    nc.scalar.activation(out=h2T[:], in_=aggT[:],
                                         func=RELU, bias=b1c[:], scale=1.0)
                    p_psum = p_ps.tile([P, P], F32, tag="pps")
                    nc.tensor.matmul(out=p_psum[:], lhsT=h2T[:], rhs=W2[:],
                                     start=True, stop=True)
                    p_sb = hp.tile([P, P], F32, tag="pout")
                    nc.vector.tensor_copy(p_sb[:], p_psum[:])
                    nc.sync.dma_start(slab2[b * P:(b + 1) * P, :], p_sb[:])

                if with_coll:
                    nc.gpsimd.collective_compute(
                        "AllGather", mybir.AluOpType.bypass,
                        replica_groups=RG,
                        ins=[slab2[:]], outs=[table2[:]])

                # ---------------- layer 3 + readout ----------------
                chunk_src = emit_gathers(table2)
                r_tiles = [r_ps.tile([P, P], F32, tag=f"rps{t}",
                                     name=f"rps{t}_{rep}")
                           for t in range(NGT)]
                for b in range(BLOCKS if with_compute else 0):
                    chunks = block_chunks(b)
                    agg = agg_ps.tile([P, P], F32, tag="aggps")
                    for j, (stream, ci) in enumerate(chunks):
                        mt, S8, col = chunk_src[(stream, ci)]
                        nc.tensor.matmul(
                            out=agg[:],
                            lhsT=S8[:][:, col * P:(col + 1) * P],
                            rhs=mt[:][:, col * P:(col + 1) * P],
                            start=(j == 0), stop=(j == len(chunks) - 1))
                    h3a = hp.tile([P, P], F32, tag="hT")
                    nc.vector.tensor_tensor(
                        out=h3a[:], in0=agg[:], in1=b2r[:],
                        op=mybir.AluOpType.add)
                    h3 = hp.tile([P, P], F32, tag="h3")
                    nc.scalar.activation(out=h3[:], in_=h3a[:],
                                         func=RELU, bias=0.0, scale=1.0)
                    # readout: one wide one-hot over all 4 graph tiles
                    Sg4 = sp.tile([P, N_GRAPHS], F32, tag="Sg4")
                    nc.vector.tensor_tensor(
                        out=Sg4[:], in0=iotg_f[:],
                        in1=gph[:][:, b:b + 1].to_broadcast([P, N_GRAPHS]),
                        op=mybir.AluOpType.is_equal)
                    for t in range(NGT):
                        nc.tensor.matmul(
                            out=r_tiles[t][:],
                            lhsT=Sg4[:][:, t * P:(t + 1) * P], rhs=h3[:],
                            start=(b == 0), stop=(b == BLOCKS - 1))

                for t in range(NGT if with_compute else 0):
                    r_sb = hp.tile([P, P], F32, tag="pout")
                    nc.vector.tensor_tensor(
                        out=r_sb[:], in0=r_tiles[t][:],
                        in1=invc[:][:, t:t + 1].to_broadcast([P, P]),
                        op=mybir.AluOpType.mult)
                    nc.sync.dma_start(partial[t * P:(t + 1) * P, :], r_sb[:])

                if with_coll and with_compute:
                    nc.gpsimd.collective_compute(
                        "AllReduce", mybir.AluOpType.add, replica_groups=RG,
                        ins=[partial[:]], outs=[summed[:]])

                # ---------------- head ----------------
                for t in range(N_GRAPHS // P if with_compute else 0):
                    rg = hp.tile([P, P], F32, tag="hT")
                    nc.sync.dma_start(rg[:], summed[t * P:(t + 1) * P, :])
                    rgT_psum = agg_ps.tile([P, P], F32, tag="aggps")
                    nc.tensor.transpose(out=rgT_psum[:], in_=rg[:],
                                        identity=ident[:])
                    rgT = hp.tile([P, P], F32, tag="pout")
                    nc.vector.tensor_copy(rgT[:], rgT_psum[:])
                    o_psum = p_ps.tile([P, N_CLASSES], F32, tag="pps")
                    nc.tensor.matmul(out=o_psum[:], lhsT=rgT[:], rhs=Wc[:],
                                     start=True, stop=True)
                    o_sb = hp.tile([P, N_CLASSES], F32, tag="osb")
                    nc.vector.tensor_tensor(out=o_sb[:], in0=o_psum[:],
                                            in1=bcr[:],
                                            op=mybir.AluOpType.add)
                    nc.sync.dma_start(out[t * P:(t + 1) * P, :], o_sb[:])

    nc.compile()
    return nc


def make_in_maps(core_arrays, invc, W0, b0, W1, b1, W2, b2, Wc, bc):
    common = dict(
        invc=np.ascontiguousarray(invc, np.float32),
        W0=np.ascontiguousarray(W0, np.float32).reshape(1, HID),
        W1=np.ascontiguousarray(W1, np.float32),
        W2=np.ascontiguousarray(W2, np.float32),
        Wc=np.ascontiguousarray(Wc, np.float32),
        b0c=np.ascontiguousarray(b0, np.float32).reshape(P, 1),
        b1c=np.ascontiguousarray(b1, np.float32).reshape(P, 1),
        b2r=np.ascontiguousarray(np.tile(np.asarray(b2, np.float32).reshape(1, HID), (P, 1))),
        bcr=np.ascontiguousarray(np.tile(np.asarray(bc, np.float32).reshape(1, N_CLASSES), (P, 1))),
    )
    in_maps = []
    for c in range(N_CORES):
        m = dict(common)
        ca = core_arrays[c]
        m["idxA"] = ca["idxA"]
        m["idxB"] = ca["idxB"]
        m["dvA"] = ca["dvA"]
        m["wA"] = ca["wA"]
        m["dvB"] = ca["dvB"]
        m["wB"] = ca["wB"]
        m["q1row"] = ca["q1row"]
        m["gphv"] = ca["gphv"]
        in_maps.append(m)
    return in_maps


_CACHE = {}


def _get_compiled(src, dst, graph_ids):
    import hashlib
    h = hashlib.md5()
    h.update(np.asarray(src).tobytes())
    h.update(np.asarray(dst).tobytes())
    h.update(np.asarray(graph_ids).tobytes())
    key = h.hexdigest()
    if key not in _CACHE:
        sched, core_arrays, invc = _prep_graph(src, dst, graph_ids)
        nc = build_nc(sched)
        _CACHE[key] = (nc, core_arrays, invc)
    return _CACHE[key]


def kernel(W0, b0, W1, b1, W2, b2, Wc, bc, src, dst, graph_ids,
           num_graphs=None, **_ignored):
    nc, core_arrays, invc = _get_compiled(src, dst, graph_ids)
    in_maps = make_in_maps(core_arrays, invc, W0, b0, W1, b1, W2, b2, Wc, bc)
    res = bass_utils.run_bass_kernel_spmd(
        nc, in_maps, core_ids=list(range(N_CORES)))
    return res.results[0]["out"]



# revision 3
# speedup vs baseline: 8.8052x; 8.8052x over previous
"""GCN classifier kernel for Trainium2 (Bass/Tile), 8-core SPMD.

Math: for each GCN layer, relu(nd * (A^T (ns * h)) @ W + b)
  == relu(sum_e w_e * p[src_e] + b) aggregated per dst, where
  p = h @ W and w_e = ns[src_e] * nd[dst_e].
Layer 0 input h0 = in_deg (rank 1), so layer 1 collapses to
  h1 = relu(q1[:, None] * W0 + b0) with q1 host-precomputable from the
  graph alone.

Device pipeline per core (owns 6272 dst nodes = 49 blocks of 128):
  L1: h1T = relu(W0^T q1 + b0) per block; p1 = h1 @ W1 -> slab1
  AllGather slab1 -> table1 (replicated 50176 x 128)
  L2: dma_gather msgs = table1[src]; aggT += msg_chunk @ S_chunk (PE);
      h2T = relu(aggT + b1); p2 = h2 @ W2 -> slab2; AllGather -> table2
  L3: gather; agg += S_chunk^T @ msg_chunk; h3 = relu(agg + b2)
      readout: r_t += S_g(t)^T @ h3 (4 global graph tiles); r_t *= 1/cnt
  AllReduce partial [512,128]; out = rg @ Wc + bc  -> [512, 10]

S chunks are one-hot(dst) * w_e matrices generated on-device by DVE
tensor_scalar(is_equal, mult) from host-prepared per-chunk columns.
dma_gather indices are int16, so the 50176-row table is addressed via two
overlapping base windows (rows 0..32767 and 17408..50175).
"""

import sys

sys.path.insert(0, "/opt/trn_rl_repo")

import numpy as np

import concourse.bass as bass
import concourse.mybir as mybir
import concourse.tile as tile
from concourse import bacc, bass_utils

P = 128
N_CORES = 8
N_NODES = 50000
N_EDGES = 800000
HID = 128
N_GRAPHS = 512
N_CLASSES = 10

NPC = 6272          # nodes per core (49 blocks of 128)
BLOCKS = NPC // P   # 49
NPAD = NPC * N_CORES  # 50176
HALF0 = 32768       # gather window 0: rows [0, 32768)
BASE1 = NPAD - 32768  # 17408; window 1: rows [17408, 50176)
GA = 8              # gather group size in chunks of 128 edges (1024 idxs = HW cap per dma_gather)
F32 = mybir.dt.float32
I16 = mybir.dt.int16
I32 = mybir.dt.int32


def _prep_graph(src, dst, graph_ids):
    """Host-side preprocessing: degrees, q1, per-core edge schedule."""
    src = np.asarray(src).astype(np.int64)
    dst = np.asarray(dst).astype(np.int64)
    graph_ids = np.asarray(graph_ids).astype(np.int64)

    in_deg = np.bincount(dst, minlength=N_NODES).astype(np.float32)
    out_deg = np.bincount(src, minlength=N_NODES).astype(np.float32)
    ns = np.maximum(out_deg, 1.0) ** -0.5
    nd = np.maximum(in_deg, 1.0) ** -0.5
    # layer-1 aggregate: q1 = nd * segsum_dst((in_deg*ns)[src])
    c0 = (in_deg * ns).astype(np.float64)
    t1 = np.bincount(dst, weights=c0[src], minlength=N_NODES)
    q1 = (nd.astype(np.float64) * t1).astype(np.float32)

    w_edge = (ns[src] * nd[dst]).astype(np.float32)

    # per-core, per-block, per-half chunk counts
    blk_of = dst % NPC  # placeholder; computed per core below
    counts = np.zeros((N_CORES, BLOCKS, 2), np.int64)
    per_core = []
    for c in range(N_CORES):
        base = c * NPC
        m = (dst >= base) & (dst < base + NPC)
        es, ed, ew = src[m], dst[m], w_edge[m]
        dloc = ed - base
        blk = dloc >> 7
        # edges with src in [BASE1, HALF0) fit either gather window; assign
        # them per block to minimize chunk padding (ceil waste)
        half = (es >= HALF0).astype(np.int64)
        over = (es >= BASE1) & (es < HALF0)
        for b in range(BLOCKS):
            mb = blk == b
            n_low = int(np.count_nonzero(mb & (es < BASE1)))
            n_over = int(np.count_nonzero(mb & over))
            n_high = int(np.count_nonzero(mb & (es >= HALF0)))
            cands = {0, n_over}
            k = (-n_low) % P
            while k <= n_over:
                cands.add(k)
                k += P
            best_x, best_cost = 0, 10**9
            for x in sorted(cands):
                cost = -(-(n_low + x) // P) + -(-(n_high + n_over - x) // P)
                if cost < best_cost:
                    best_cost, best_x = cost, x
            if best_x < n_over:
                idxs_over = np.nonzero(mb & over)[0]
                half[idxs_over[best_x:]] = 1
        order = np.lexsort((es, half, blk))
        es, dloc, ew, blk, half = (
            es[order], dloc[order], ew[order], blk[order], half[order])
        for b in range(BLOCKS):
            mb = blk == b
            counts[c, b, 0] = np.count_nonzero(mb & (half == 0))
            counts[c, b, 1] = np.count_nonzero(mb & (half == 1))
        per_core.append((es, dloc, ew, blk, half))

    K0 = np.maximum(1, np.ceil(counts[:, :, 0] / P).max(axis=0).astype(np.int64))
    K1 = np.ceil(counts[:, :, 1] / P).max(axis=0).astype(np.int64)
    KA = int(K0.sum())
    KB = int(K1.sum())

    core_arrays = []
    for c in range(N_CORES):
        es, dloc, ew, blk, half = per_core[c]
        base = c * NPC
        idxA = np.zeros(KA * P, np.int32)
        dvA = np.zeros(KA * P, np.float32)
        wA = np.zeros(KA * P, np.float32)
        idxB = np.zeros(KB * P, np.int32)
        dvB = np.zeros(KB * P, np.float32)
        wB = np.zeros(KB * P, np.float32)
        offA = 0
        offB = 0
        for b in range(BLOCKS):
            for h, (idxs, dvs, ws, K, off) in enumerate((
                (idxA, dvA, wA, int(K0[b]), offA),
                (idxB, dvB, wB, int(K1[b]), offB),
            )):
                m = (blk == b) & (half == h)
                n = int(np.count_nonzero(m))
                assert n <= K * P
                sl = slice(off, off + n)
                idxs[sl] = es[m] - (0 if h == 0 else BASE1)
                dvs[sl] = (dloc[m] - b * P).astype(np.float32)
                ws[sl] = ew[m]
                # padding stays idx=0, dstv=0, w=0 (contributes 0 via S)
            offA += int(K0[b]) * P
            offB += int(K1[b]) * P

        def idx_layout(v):
            # index i -> partition i%16 (replicated x8), column i//16
            r = v.astype(np.int16).reshape(-1, 16).T  # [16, L/16]
            return np.tile(r, (8, 1)).copy()  # [128, L/16]

        def col_layout(v):
            return np.ascontiguousarray(v.reshape(-1, P).T)  # [128, K]

        own = np.arange(base, base + NPC)
        q1row = np.zeros((1, NPC), np.float32)
        real = own < N_NODES
        q1row[0, real] = q1[own[real]]
        gph = np.full(NPC, -1.0, np.float32)
        gph[real] = graph_ids[own[real]].astype(np.float32)

        core_arrays.append(dict(
            idxA=idx_layout(idxA), idxB=idx_layout(idxB),
            dvA=col_layout(dvA), wA=col_layout(wA),
            dvB=col_layout(dvB), wB=col_layout(wB),
            q1row=q1row,
            gphv=np.ascontiguousarray(gph.reshape(BLOCKS, P).T),
        ))

    cnt = np.bincount(graph_ids, minlength=N_GRAPHS).astype(np.float32)
    invc = (1.0 / np.maximum(cnt, 1.0)).reshape(N_GRAPHS // P, P).T  # [128, 4]
    invc = np.ascontiguousarray(invc)

    sched = dict(K0=K0, K1=K1, KA=KA, KB=KB)
    return sched, core_arrays, invc


def build_nc(sched, reps=1, with_coll=True, with_gather=True,
             with_sgen=True, with_compute=True, msg_bufs=16, sgen_bufs=12,
             hbuf_bufs=6):
    """Build and compile the 8-core SPMD Bass program.

    reps>1 repeats the whole pipeline inside one NEFF (for timing via
    slope); with_coll/with_gather=False drop those phases (timing only —
    results are garbage)."""
    K0, K1, KA, KB = sched["K0"], sched["K1"], sched["KA"], sched["KB"]
    NGT = N_GRAPHS // P  # 4

    nc = bacc.Bacc("TRN2", target_bir_lowering=False, debug=False,
                   num_devices=N_CORES, num_swdge_queues=4)

    def inp(name, shape, dt=F32):
        return nc.dram_tensor(name, list(shape), dt, kind="ExternalInput").ap()

    d_idxA = inp("idxA", [P, KA * 8], I16)
    d_idxB = inp("idxB", [P, max(KB, 1) * 8], I16)
    d_dvA = inp("dvA", [P, KA])
    d_wA = inp("wA", [P, KA])
    d_dvB = inp("dvB", [P, max(KB, 1)])
    d_wB = inp("wB", [P, max(KB, 1)])
    d_q1 = inp("q1row", [1, NPC])
    d_gph = inp("gphv", [P, BLOCKS])
    d_invc = inp("invc", [P, NGT])
    d_W0 = inp("W0", [1, HID])
    d_W1 = inp("W1", [HID, HID])
    d_W2 = inp("W2", [HID, HID])
    d_Wc = inp("Wc", [HID, N_CLASSES])
    d_b0c = inp("b0c", [P, 1])
    d_b1c = inp("b1c", [P, 1])
    d_b2r = inp("b2r", [P, HID])
    d_bcr = inp("bcr", [P, N_CLASSES])

    out = nc.dram_tensor("out", [N_GRAPHS, N_CLASSES], F32,
                         kind="ExternalOutput").ap()

    slab1 = nc.dram_tensor("slab1", [NPC, HID], F32, kind="Internal").ap()
    slab2 = nc.dram_tensor("slab2", [NPC, HID], F32, kind="Internal").ap()
    table1 = nc.dram_tensor("table1", [NPAD, HID], F32, kind="Internal",
                            addr_space="Shared").ap()
    table2 = nc.dram_tensor("table2", [NPAD, HID], F32, kind="Internal",
                            addr_space="Shared").ap()
    partial = nc.dram_tensor("partial", [N_GRAPHS, HID], F32,
                             kind="Internal").ap()
    summed = nc.dram_tensor("summed", [N_GRAPHS, HID], F32, kind="Internal",
                            addr_space="Shared").ap()

    RG = [list(range(N_CORES))]

    # block -> chunk ranges in streams A and B
    offA = np.concatenate([[0], np.cumsum(K0)]).astype(int)
    offB = np.concatenate([[0], np.cumsum(K1)]).astype(int)

    with tile.TileContext(nc) as tc:
        with tc.tile_pool(name="const", bufs=1) as cp, \
             tc.tile_pool(name="msg", bufs=msg_bufs) as mp, \
             tc.tile_pool(name="sgen", bufs=sgen_bufs) as sp, \
             tc.tile_pool(name="hbuf", bufs=hbuf_bufs) as hp, \
             tc.tile_pool(name="agg_ps", bufs=2, space="PSUM") as agg_ps, \
             tc.tile_pool(name="p_ps", bufs=2, space="PSUM") as p_ps, \
             tc.tile_pool(name="r_ps", bufs=1, space="PSUM") as r_ps:

            def load_const(ap_in, shape, dt=F32):
                t = cp.tile(list(shape), dt, tag=ap_in.name)
                nc.sync.dma_start(t[:], ap_in[:])
                return t

            idxA = load_const(d_idxA, [P, KA * 8], I16)
            idxB = load_const(d_idxB, [P, max(KB, 1) * 8], I16)
            dvA = load_const(d_dvA, [P, KA])
            wA = load_const(d_wA, [P, KA])
            dvB = load_const(d_dvB, [P, max(KB, 1)])
            wB = load_const(d_wB, [P, max(KB, 1)])
            q1r = load_const(d_q1, [1, NPC])
            gph = load_const(d_gph, [P, BLOCKS])
            invc = load_const(d_invc, [P, NGT])
            W0 = load_const(d_W0, [1, HID])
            W1 = load_const(d_W1, [HID, HID])
            W2 = load_const(d_W2, [HID, HID])
            Wc = load_const(d_Wc, [HID, N_CLASSES])
            b0c = load_const(d_b0c, [P, 1])
            b1c = load_const(d_b1c, [P, 1])
            b2r = load_const(d_b2r, [P, HID])
            bcr = load_const(d_bcr, [P, N_CLASSES])

            iota_i = cp.tile([P, P], I32, tag="iota_i")
            nc.gpsimd.iota(iota_i[:], pattern=[[1, P]], base=0,
                           channel_multiplier=0)
            iota_f = cp.tile([P, P], F32, tag="iota_f")
            nc.vector.tensor_copy(iota_f[:], iota_i[:])
            iotg_i = cp.tile([P, N_GRAPHS], I32, tag="iotg_i")
            nc.gpsimd.iota(iotg_i[:], pattern=[[1, N_GRAPHS]], base=0,
                           channel_multiplier=0)
            iotg_f = cp.tile([P, N_GRAPHS], F32, tag="iotg_f")
            nc.vector.tensor_copy(iotg_f[:], iotg_i[:])

            ident = cp.tile([P, P], F32, tag="ident")
            from concourse.masks import make_identity
            make_identity(nc, ident[:])

            RELU = mybir.ActivationFunctionType.Relu

            # Global gather-instruction counter: msg pool slot = count %
            # msg_bufs, SWDGE queue = count % 4. With msg_bufs % 4 == 0 each
            # pool slot always sees the same queue, which CoreSim requires
            # (semaphores are locked to one SWDGE queue).
            gather_count = [0]

            # block -> chunk list over both streams
            def block_chunks(b):
                res = []
                for ca in range(offA[b], offA[b + 1]):
                    res.append(("A", ca))
                for cb in range(offB[b], offB[b + 1]):
                    res.append(("B", cb))
                return res

            for rep in range(reps):
                # ---------------- layer 1 ----------------
                for k in range(BLOCKS):
                    h1T_psum = agg_ps.tile([P, P], F32, tag="aggps")
                    nc.tensor.matmul(
                        out=h1T_psum[:], lhsT=W0[:],
                        rhs=q1r[:][:, k * P:(k + 1) * P],
                        start=True, stop=True)
                    h1T = hp.tile([P, P], F32, tag="hT")
                    nc.scalar.activation(out=h1T[:], in_=h1T_psum[:],
                                         func=RELU, bias=b0c[:], scale=1.0)
                    p_psum = p_ps.tile([P, P], F32, tag="pps")
                    nc.tensor.matmul(out=p_psum[:], lhsT=h1T[:], rhs=W1[:],
                                     start=True, stop=True)
                    p_sb = hp.tile([P, P], F32, tag="pout")
                    nc.vector.tensor_copy(p_sb[:], p_psum[:])
                    nc.sync.dma_start(slab1[k * P:(k + 1) * P, :], p_sb[:])

                if with_coll:
                    nc.gpsimd.collective_compute(
                        "AllGather", mybir.AluOpType.bypass,
                        replica_groups=RG,
                        ins=[slab1[:]], outs=[table1[:]])

                def emit_gathers(table_ap):
                    """Gather + S-gen per group, block-sorted across streams.

                    Returns chunk -> (msg tile, S8 tile, col). Pool executes
                    gathers in order, so groups are emitted in first-use
                    (block) order to avoid msg-slot deadlock.
                    """
                    chunk_src = {}
                    groups = []
                    blockA = np.searchsorted(offA[1:], np.arange(KA),
                                             side="right")
                    blockB = np.searchsorted(offB[1:], np.arange(max(KB, 1)),
                                             side="right")
                    for stream, K, idx_t, blk_of in (
                            ("A", KA, idxA, blockA), ("B", KB, idxB, blockB)):
                        base_ap = (table_ap[0:HALF0, :] if stream == "A"
                                   else table_ap[BASE1:NPAD, :])
                        g0 = 0
                        while g0 < K:
                            ln = min(GA, K - g0)
                            groups.append(
                                (int(blk_of[g0]), stream, g0, ln, base_ap,
                                 idx_t))
                            g0 += ln
                    groups.sort(key=lambda g: (g[0], g[1]))
                    for _fb, stream, g0, ln, base_ap, idx_t in groups:
                        gi = gather_count[0]
                        gather_count[0] += 1
                        mt = mp.tile([P, GA * P], F32, tag="msg")
                        out_ap = mt[:][:, :ln * P].rearrange(
                            "p (a b) -> p a b", b=P)
                        if with_gather:
                            nc.gpsimd.dma_gather(
                                out_ap=out_ap, in_ap=base_ap,
                                idxs_ap=idx_t[:][:, g0 * 8:(g0 + ln) * 8],
                                num_idxs=ln * P, num_idxs_reg=ln * P,
                                elem_size=HID, queue_num=gi % 4)
                        # S for the whole group: two wide DVE ops
                        dv, w = (dvA, wA) if stream == "A" else (dvB, wB)
                        S8 = sp.tile([P, GA * P], F32, tag="S8")
                        s_ap = S8[:][:, :ln * P].rearrange(
                            "p (a b) -> p a b", b=P)
                        if with_sgen:
                            io8 = iota_f[:].unsqueeze(1).broadcast_to(
                                [P, ln, P])
                            dv8 = dv[:][:, g0:g0 + ln].unsqueeze(2). \
                                broadcast_to([P, ln, P])
                            w8 = w[:][:, g0:g0 + ln].unsqueeze(2). \
                                broadcast_to([P, ln, P])
                            nc.vector.tensor_tensor(
                                out=s_ap, in0=io8, in1=dv8,
                                op=mybir.AluOpType.is_equal)
                            nc.vector.tensor_tensor(
                                out=s_ap, in0=s_ap, in1=w8,
                                op=mybir.AluOpType.mult)
                        for j in range(ln):
                            chunk_src[(stream, g0 + j)] = (mt, S8, j)
                    return chunk_src

                # ---------------- layer 2 ----------------
                chunk_src = emit_gathers(table1)
                for b in range(BLOCKS if with_compute else 0):
                    chunks = block_chunks(b)
                    aggT = agg_ps.tile([P, P], F32, tag="aggps")
                    for j, (stream, ci) in enumerate(chunks):
                        mt, S8, col = chunk_src[(stream, ci)]
                        nc.tensor.matmul(
                            out=aggT[:],
                            lhsT=mt[:][:, col * P:(col + 1) * P],
                            rhs=S8[:][:, col * P:(col + 1) * P],
                            start=(j == 0), stop=(j == len(chunks) - 1))
                    h2T = hp.tile([P, P], F32, tag="hT")
                    nc.scalar.activation(out=h2T[:], in_=aggT[:],
                                         func=RELU, bias=b1c[:], scale=1.0)
                    p_psum = p_ps.tile([P, P], F32, tag="pps")
                    nc.tensor.matmul(out=p_psum[:], lhsT=h2T[:], rhs=W2[:],
                                     start=True, stop=True)
                    p_sb = hp.tile([P, P], F32, tag="pout")
                    nc.vector.tensor_copy(p_sb[:], p_psum[:])
                    nc.sync.dma_start(slab2[b * P:(b + 1) * P, :], p_sb[:])

                if with_coll:
                    nc.gpsimd.collective_compute(
                        "AllGather", mybir.AluOpType.bypass,
                        replica_groups=RG,
                        ins=[slab2[:]], outs=[table2[:]])

                # ---------------- layer 3 + readout ----------------
                chunk_src = emit_gathers(table2)
                r_tiles = [r_ps.tile([P, P], F32, tag=f"rps{t}",
                                     name=f"rps{t}_{rep}")
                           for t in range(NGT)]
                for b in range(BLOCKS if with_compute else 0):
                    chunks = block_chunks(b)
                    agg = agg_ps.tile([P, P], F32, tag="aggps")
                    for j, (stream, ci) in enumerate(chunks):
                        mt, S8, col = chunk_src[(stream, ci)]
                        nc.tensor.matmul(
                            out=agg[:],
                            lhsT=S8[:][:, col * P:(col + 1) * P],
                            rhs=mt[:][:, col * P:(col + 1) * P],
                            start=(j == 0), stop=(j == len(chunks) - 1))
                    h3a = hp.tile([P, P], F32, tag="hT")
                    nc.vector.tensor_tensor(
                        out=h3a[:], in0=agg[:], in1=b2r[:],
                        op=mybir.AluOpType.add)
                    h3 = hp.tile([P, P], F32, tag="h3")
                    nc.scalar.activation(out=h3[:], in_=h3a[:],
                                         func=RELU, bias=0.0, scale=1.0)
                    # readout: one wide one-hot over all 4 graph tiles
                    Sg4 = sp.tile([P, N_GRAPHS], F32, tag="Sg4")
                    nc.vector.tensor_tensor(
                        out=Sg4[:], in0=iotg_f[:],
                        in1=gph[:][:, b:b + 1].to_broadcast([P, N_GRAPHS]),
                        op=mybir.AluOpType.is_equal)
                    for t in range(NGT):
                        nc.tensor.matmul(
                            out=r_tiles[t][:],
                            lhsT=Sg4[:][:, t * P:(t + 1) * P], rhs=h3[:],
                            start=(b == 0), stop=(b == BLOCKS - 1))

                for t in range(NGT if with_compute else 0):
                    r_sb = hp.tile([P, P], F32, tag="pout")
                    nc.vector.tensor_tensor(
                        out=r_sb[:], in0=r_tiles[t][:],
                        in1=invc[:][:, t:t + 1].to_broadcast([P, P]),
                        op=mybir.AluOpType.mult)
                    nc.sync.dma_start(partial[t * P:(t + 1) * P, :], r_sb[:])

                if with_coll and with_compute:
                    nc.gpsimd.collective_compute(
                        "AllReduce", mybir.AluOpType.add, replica_groups=RG,
                        ins=[partial[:]], outs=[summed[:]])

                # ---------------- head ----------------
                for t in range(N_GRAPHS // P if with_compute else 0):
                    rg = hp.tile([P, P], F32, tag="hT")
                    nc.sync.dma_start(rg[:], summed[t * P:(t + 1) * P, :])
                    rgT_psum = agg_ps.tile([P, P], F32, tag="aggps")
                    nc.tensor.transpose(out=rgT_psum[:], in_=rg[:],
                                        identity=ident[:])
                    rgT = hp.tile([P, P], F32, tag="pout")
                    nc.vector.tensor_copy(rgT[:], rgT_psum[:])
                    o_psum = p_ps.tile([P, N_CLASSES], F32, tag="pps")
                    nc.tensor.matmul(out=o_psum[:], lhsT=rgT[:], rhs=Wc[:],
                                     start=True, stop=True)
                    o_sb = hp.tile([P, N_CLASSES], F32, tag="osb")
                    nc.vector.tensor_tensor(out=o_sb[:], in0=o_psum[:],
                                            in1=bcr[:],
                                            op=mybir.AluOpType.add)
                    nc.sync.dma_start(out[t * P:(t + 1) * P, :], o_sb[:])

    nc.compile()
    return nc


def make_in_maps(core_arrays, invc, W0, b0, W1, b1, W2, b2, Wc, bc):
    common = dict(
        invc=np.ascontiguousarray(invc, np.float32),
        W0=np.ascontiguousarray(W0, np.float32).reshape(1, HID),
        W1=np.ascontiguousarray(W1, np.float32),
        W2=np.ascontiguousarray(W2, np.float32),
        Wc=np.ascontiguousarray(Wc, np.float32),
        b0c=np.ascontiguousarray(b0, np.float32).reshape(P, 1),
        b1c=np.ascontiguousarray(b1, np.float32).reshape(P, 1),
        b2r=np.ascontiguousarray(np.tile(np.asarray(b2, np.float32).reshape(1, HID), (P, 1))),
        bcr=np.ascontiguousarray(np.tile(np.asarray(bc, np.float32).reshape(1, N_CLASSES), (P, 1))),
    )
    in_maps = []
    for c in range(N_CORES):
        m = dict(common)
        ca = core_arrays[c]
        m["idxA"] = ca["idxA"]
        m["idxB"] = ca["idxB"]
        m["dvA"] = ca["dvA"]
        m["wA"] = ca["wA"]
        m["dvB"] = ca["dvB"]
        m["wB"] = ca["wB"]
        m["q1row"] = ca["q1row"]
        m["gphv"] = ca["gphv"]
        in_maps.append(m)
    return in_maps


_CACHE = {}


def _get_compiled(src, dst, graph_ids):
    import hashlib
    h = hashlib.md5()
    h.update(np.asarray(src).tobytes())
    h.update(np.asarray(dst).tobytes())
    h.update(np.asarray(graph_ids).tobytes())
    key = h.hexdigest()
    if key not in _CACHE:
        sched, core_arrays, invc = _prep_graph(src, dst, graph_ids)
        nc = build_nc(sched)
        _CACHE[key] = (nc, core_arrays, invc)
    return _CACHE[key]


def kernel(W0, b0, W1, b1, W2, b2, Wc, bc, src, dst, graph_ids,
           num_graphs=None, **_ignored):
    nc, core_arrays, invc = _get_compiled(src, dst, graph_ids)
    in_maps = make_in_maps(core_arrays, invc, W0, b0, W1, b1, W2, b2, Wc, bc)
    res = bass_utils.run_bass_kernel_spmd(
        nc, in_maps, core_ids=list(range(N_CORES)))
    return res.results[0]["out"]



# revision 5
# speedup vs baseline: 10.9881x; 1.2479x over previous
"""GCN classifier kernel v8: v5 + wide-group S-gen, single AllGather.

Key structural changes vs baseline:
- Layer-1 output h1 = relu(q1 W0 + b0) is rank-1 in the per-node scalar q1,
  so every core computes the FULL h1 table locally (392 blocks of a K=2
  matmul + relu) -> no table1 AllGather at all. The W1 projection moves to
  after the L2 aggregation (associativity), so table1 stores h1 itself.
- All gather tables / matmul operands in fp16 (PE 1 cyc/row vs fp32's 4).
- S one-hot chunks generated by two wide DVE ops per 8-chunk gather group
  (is_equal then mult over [128, 1024] broadcast APs) -- per-chunk DVE
  instructions measured ~400us slower on HW.
- 1/cnt readout normalization folded into h3's activation scale; per-graph
  readout is one wide [dst,512] matmul per block accumulating rT [f, 512g];
  head consumes rT directly as lhsT (no transposes).
- L3 bias b2 added by an extra K=1 matmul chunk (ones x b2row) on PE.

Pipeline per core (owns 6272 dst nodes = 49 blocks of 128):
  L1: for all 392 blocks: E4 = (qm-slice)^T @ W0b0 (K=2); h1 = relu(E4)
      (grouped x4 in one PSUM bank); DMA -> local table1 (no collective).
  L2: dma_gather msg = table1[src]; S via fused DVE op;
      aggT_h += msg_chunk^T? (lhsT=msg, rhs=S) per chunk;
      z = W1^T @ aggT_h; h2T = relu(z + b1); p2 = h2T^T @ W2 -> slab2.
  AllGather slab2 -> table2 (replicated [50176, 128] bf16)
  L3: gather; agg += S^T @ msg; agg += ones x b2row;
      h3 = relu(agg) * invc_dst (act scale); rT += h3^T @ Sg4 [f, 512]
  AllReduce partial rT; out = rT_tile^T @ Wc + bc -> [512, 10]
"""

import sys

sys.path.insert(0, "/opt/trn_rl_repo")

import numpy as np

import concourse.bass as bass
import concourse.mybir as mybir
import concourse.tile as tile
from concourse import bacc, bass_utils

P = 128
N_CORES = 8
N_NODES = 50000
N_EDGES = 800000
HID = 128
N_GRAPHS = 512
N_CLASSES = 10

NPC = 6272          # nodes per core (49 blocks of 128)
BLOCKS = NPC // P   # 49
NPAD = NPC * N_CORES  # 50176
NBLK = NPAD // P    # 392 blocks over all nodes
HALF0 = 32768       # gather window 0: rows [0, 32768)
BASE1 = NPAD - 32768  # 17408; window 1: rows [17408, 50176)
GA = 8              # gather group size in chunks (1024 idxs = HW cap)
F32 = mybir.dt.float32
F16 = mybir.dt.float16
I16 = mybir.dt.int16
I32 = mybir.dt.int32

TDT = F16           # gather table dtype


def _prep_graph(src, dst, graph_ids):
    """Host-side preprocessing: degrees, q1, per-core edge schedule."""
    src = np.asarray(src).astype(np.int64)
    dst = np.asarray(dst).astype(np.int64)
    graph_ids = np.asarray(graph_ids).astype(np.int64)

    in_deg = np.bincount(dst, minlength=N_NODES).astype(np.float32)
    out_deg = np.bincount(src, minlength=N_NODES).astype(np.float32)
    ns = np.maximum(out_deg, 1.0) ** -0.5
    nd = np.maximum(in_deg, 1.0) ** -0.5
    # layer-1 aggregate: q1 = nd * segsum_dst((in_deg*ns)[src])
    c0 = (in_deg * ns).astype(np.float64)
    t1 = np.bincount(dst, weights=c0[src], minlength=N_NODES)
    q1 = (nd.astype(np.float64) * t1).astype(np.float32)

    w_edge = (ns[src] * nd[dst]).astype(np.float32)

    # per-core, per-block, per-half chunk counts
    counts = np.zeros((N_CORES, BLOCKS, 2), np.int64)
    per_core = []
    for c in range(N_CORES):
        base = c * NPC
        m = (dst >= base) & (dst < base + NPC)
        es, ed, ew = src[m], dst[m], w_edge[m]
        dloc = ed - base
        blk = dloc >> 7
        # edges with src in [BASE1, HALF0) fit either gather window; assign
        # them per block to minimize chunk padding (ceil waste)
        half = (es >= HALF0).astype(np.int64)
        over = (es >= BASE1) & (es < HALF0)
        for b in range(BLOCKS):
            mb = blk == b
            n_low = int(np.count_nonzero(mb & (es < BASE1)))
            n_over = int(np.count_nonzero(mb & over))
            cands = {0, n_over}
            k = (-n_low) % P
            while k <= n_over:
                cands.add(k)
                k += P
            n_high = int(np.count_nonzero(mb & (es >= HALF0)))
            best_x, best_cost = 0, 10**9
            for x in sorted(cands):
                cost = -(-(n_low + x) // P) + -(-(n_high + n_over - x) // P)
                if cost < best_cost:
                    best_cost, best_x = cost, x
            if best_x < n_over:
                idxs_over = np.nonzero(mb & over)[0]
                half[idxs_over[best_x:]] = 1
        order = np.lexsort((es, half, blk))
        es, dloc, ew, blk, half = (
            es[order], dloc[order], ew[order], blk[order], half[order])
        for b in range(BLOCKS):
            mb = blk == b
            counts[c, b, 0] = np.count_nonzero(mb & (half == 0))
            counts[c, b, 1] = np.count_nonzero(mb & (half == 1))
        per_core.append((es, dloc, ew, blk, half))

    K0 = np.maximum(1, np.ceil(counts[:, :, 0] / P).max(axis=0).astype(np.int64))
    K1 = np.ceil(counts[:, :, 1] / P).max(axis=0).astype(np.int64)
    KA = int(K0.sum())
    KB = int(K1.sum())

    core_arrays = []
    for c in range(N_CORES):
        es, dloc, ew, blk, half = per_core[c]
        base = c * NPC
        idxA = np.zeros(KA * P, np.int32)
        dvA = np.zeros(KA * P, np.float32)
        wA = np.zeros(KA * P, np.float32)
        idxB = np.zeros(KB * P, np.int32)
        dvB = np.zeros(KB * P, np.float32)
        wB = np.zeros(KB * P, np.float32)
        offA = 0
        offB = 0
        for b in range(BLOCKS):
            for h, (idxs, dvs, ws, K, off) in enumerate((
                (idxA, dvA, wA, int(K0[b]), offA),
                (idxB, dvB, wB, int(K1[b]), offB),
            )):
                m = (blk == b) & (half == h)
                n = int(np.count_nonzero(m))
                assert n <= K * P
                sl = slice(off, off + n)
                idxs[sl] = es[m] - (0 if h == 0 else BASE1)
                dvs[sl] = (dloc[m] - b * P).astype(np.float32)
                ws[sl] = ew[m]
                # padding stays idx=0, dstv=0, w=0 (contributes 0 via S)
            offA += int(K0[b]) * P
            offB += int(K1[b]) * P

        def idx_layout(v):
            # index i -> partition i%16, column i//16 (replicated to 128
            # partitions on device)
            return np.ascontiguousarray(
                v.astype(np.int16).reshape(-1, 16).T)  # [16, L/16]

        def col_layout(v, dt=np.float32):
            return np.ascontiguousarray(v.reshape(-1, P).T.astype(dt))

        own = np.arange(base, base + NPC)
        real = own < N_NODES
        gph = np.full(NPC, -1.0, np.float32)
        gph[real] = graph_ids[own[real]].astype(np.float32)

        cnt = np.bincount(graph_ids, minlength=N_GRAPHS).astype(np.float32)
        inv_of_dst = np.zeros(NPC, np.float32)
        inv_of_dst[real] = 1.0 / np.maximum(cnt[graph_ids[own[real]]], 1.0)

        core_arrays.append(dict(
            idxA=idx_layout(idxA), idxB=idx_layout(idxB),
            dvA=col_layout(dvA, np.float16), wA=col_layout(wA, np.float16),
            dvB=col_layout(dvB, np.float16), wB=col_layout(wB, np.float16),
            gphv=np.ascontiguousarray(gph.reshape(BLOCKS, P).T),
            invd=np.ascontiguousarray(
                inv_of_dst.reshape(BLOCKS, P).T),  # [128, 49] f32
        ))

    # qm [2, NPAD]: row0 = q1 (padded), row1 = 1.0 -- replicated input
    qm = np.zeros((2, NPAD), np.float32)
    qm[0, :N_NODES] = q1
    qm[1, :] = 1.0
    qm = qm.astype(np.float16)

    sched = dict(K0=K0, K1=K1, KA=KA, KB=KB)
    return sched, core_arrays, qm


def build_nc(sched, reps=1, with_coll=True, with_gather=True,
             with_sgen=True, with_compute=True, msg_bufs=16, sgen_bufs=16,
             hbuf_bufs=6, l1_bufs=4):
    """Build and compile the 8-core SPMD Bass program."""
    K0, K1, KA, KB = sched["K0"], sched["K1"], sched["KA"], sched["KB"]
    KBx = max(KB, 1)
    NGT = N_GRAPHS // P  # 4

    nc = bacc.Bacc("TRN2", target_bir_lowering=False, debug=False,
                   num_devices=N_CORES, num_swdge_queues=4)

    def inp(name, shape, dt=F32):
        return nc.dram_tensor(name, list(shape), dt, kind="ExternalInput").ap()

    d_idxA = inp("idxA", [16, KA * 8], I16)
    d_idxB = inp("idxB", [16, KBx * 8], I16)
    d_dvA = inp("dvA", [P, KA], F16)
    d_wA = inp("wA", [P, KA], F16)
    d_dvB = inp("dvB", [P, KBx], F16)
    d_wB = inp("wB", [P, KBx], F16)
    d_qm = inp("qm", [2, NPAD], F16)
    d_gph = inp("gphv", [P, BLOCKS])
    d_invd = inp("invd", [P, BLOCKS])
    d_W0b0 = inp("W0b0", [2, HID], F16)
    d_W1 = inp("W1b", [HID, HID], F16)
    d_W2 = inp("W2b", [HID, HID], F16)
    d_Wc = inp("Wc", [HID, N_CLASSES])
    d_b1c = inp("b1c", [P, 1])
    d_b2row = inp("b2row", [1, HID], F16)
    d_bcr = inp("bcr", [P, N_CLASSES])

    out = nc.dram_tensor("out", [N_GRAPHS, N_CLASSES], F32,
                         kind="ExternalOutput").ap()

    table1 = nc.dram_tensor("table1", [NPAD, HID], TDT, kind="Internal").ap()
    slab2 = nc.dram_tensor("slab2", [NPC, HID], TDT, kind="Internal").ap()
    table2 = nc.dram_tensor("table2", [NPAD, HID], TDT, kind="Internal",
                            addr_space="Shared").ap()
    partial = nc.dram_tensor("partial", [HID, N_GRAPHS], F32,
                             kind="Internal").ap()
    summed = nc.dram_tensor("summed", [HID, N_GRAPHS], F32, kind="Internal",
                            addr_space="Shared").ap()

    RG = [list(range(N_CORES))]

    # block -> chunk ranges in streams A and B
    offA = np.concatenate([[0], np.cumsum(K0)]).astype(int)
    offB = np.concatenate([[0], np.cumsum(K1)]).astype(int)

    with tile.TileContext(nc) as tc:
        with tc.tile_pool(name="const", bufs=1) as cp, \
             tc.tile_pool(name="qmp", bufs=3) as qmp, \
             tc.tile_pool(name="msg", bufs=msg_bufs) as mp, \
             tc.tile_pool(name="sgen", bufs=sgen_bufs) as sp, \
             tc.tile_pool(name="hbuf", bufs=hbuf_bufs) as hp, \
             tc.tile_pool(name="h1x", bufs=3) as h1p, \
             tc.tile_pool(name="l1_ps", bufs=2, space="PSUM") as l1_ps, \
             tc.tile_pool(name="agg_ps", bufs=2, space="PSUM") as agg_ps, \
             tc.tile_pool(name="p_ps", bufs=2, space="PSUM") as p_ps, \
             tc.tile_pool(name="r_ps", bufs=1, space="PSUM") as r_ps:

            def load_const(ap_in, shape, dt=F32):
                t = cp.tile(list(shape), dt, tag=ap_in.name)
                nc.sync.dma_start(t[:], ap_in[:])
                return t

            idxA = cp.tile([P, KA * 8], I16, tag="idxA")
            idxB = cp.tile([P, KBx * 8], I16, tag="idxB")
            for r in range(8):
                nc.sync.dma_start(idxA[:][r * 16:(r + 1) * 16, :], d_idxA[:])
                nc.sync.dma_start(idxB[:][r * 16:(r + 1) * 16, :], d_idxB[:])
            dvA = load_const(d_dvA, [P, KA], F16)
            wA = load_const(d_wA, [P, KA], F16)
            dvB = load_const(d_dvB, [P, KBx], F16)
            wB = load_const(d_wB, [P, KBx], F16)
            gph = load_const(d_gph, [P, BLOCKS])
            invd = load_const(d_invd, [P, BLOCKS])
            W0b0 = load_const(d_W0b0, [2, HID], F16)
            W1b = load_const(d_W1, [HID, HID], F16)
            W2b = load_const(d_W2, [HID, HID], F16)
            Wc = load_const(d_Wc, [HID, N_CLASSES])
            b1c = load_const(d_b1c, [P, 1])
            b2row = load_const(d_b2row, [1, HID], F16)
            bcr = load_const(d_bcr, [P, N_CLASSES])

            ones1 = cp.tile([1, P], F16, tag="ones1")
            nc.vector.memset(ones1[:], 1.0)

            iota_i = cp.tile([P, P], I32, tag="iota_i")
            nc.gpsimd.iota(iota_i[:], pattern=[[1, P]], base=0,
                           channel_multiplier=0)
            iota_b = cp.tile([P, P], F16, tag="iota_b")
            nc.vector.tensor_copy(iota_b[:], iota_i[:])
            iotg_i = cp.tile([P, N_GRAPHS], I32, tag="iotg_i")
            nc.gpsimd.iota(iotg_i[:], pattern=[[1, N_GRAPHS]], base=0,
                           channel_multiplier=0)
            iotg_f = cp.tile([P, N_GRAPHS], F16, tag="iotg_f")
            nc.vector.tensor_copy(iotg_f[:], iotg_i[:])

            RELU = mybir.ActivationFunctionType.Relu

            # block -> chunk list over both streams
            def block_chunks(b):
                res = []
                for ca in range(offA[b], offA[b + 1]):
                    res.append(("A", ca))
                for cb in range(offB[b], offB[b + 1]):
                    res.append(("B", cb))
                return res

            # Global gather-instruction counter: msg pool slot = count %
            # msg_bufs, SWDGE queue = count % 4 stays consistent per slot.
            gather_count = [0]

            def emit_gathers(table_ap):
                """Gather msgs per group, block-sorted across streams."""
                chunk_src = {}
                groups = []
                blockA = np.searchsorted(offA[1:], np.arange(KA),
                                         side="right")
                blockB = np.searchsorted(offB[1:], np.arange(KBx),
                                         side="right")
                for stream, K, idx_t, blk_of in (
                        ("A", KA, idxA, blockA), ("B", KB, idxB, blockB)):
                    base_ap = (table_ap[0:HALF0, :] if stream == "A"
                               else table_ap[BASE1:NPAD, :])
                    g0 = 0
                    while g0 < K:
                        ln = min(GA, K - g0)
                        groups.append(
                            (int(blk_of[g0]), stream, g0, ln, base_ap,
                             idx_t))
                        g0 += ln
                groups.sort(key=lambda g: (g[0], g[1]))
                for _fb, stream, g0, ln, base_ap, idx_t in groups:
                    gi = gather_count[0]
                    gather_count[0] += 1
                    mt = mp.tile([P, GA * P], TDT, tag="msg")
                    out_ap = mt[:][:, :ln * P].rearrange(
                        "p (a b) -> p a b", b=P)
                    if with_gather:
                        nc.gpsimd.dma_gather(
                            out_ap=out_ap, in_ap=base_ap,
                            idxs_ap=idx_t[:][:, g0 * 8:(g0 + ln) * 8],
                            num_idxs=ln * P, num_idxs_reg=ln * P,
                            elem_size=HID, queue_num=gi % 4)
                    dv, w = (dvA, wA) if stream == "A" else (dvB, wB)
                    S8 = sp.tile([P, GA * P], F16, tag="S8")
                    s_ap = S8[:][:, :ln * P].rearrange(
                        "p (a b) -> p a b", b=P)
                    if with_sgen:
                        io8 = iota_b[:].unsqueeze(1).broadcast_to(
                            [P, ln, P])
                        dv8 = dv[:][:, g0:g0 + ln].unsqueeze(2). \
                            broadcast_to([P, ln, P])
                        w8 = w[:][:, g0:g0 + ln].unsqueeze(2). \
                            broadcast_to([P, ln, P])
                        nc.vector.tensor_tensor(
                            out=s_ap, in0=io8, in1=dv8,
                            op=mybir.AluOpType.is_equal)
                        nc.vector.tensor_tensor(
                            out=s_ap, in0=s_ap, in1=w8,
                            op=mybir.AluOpType.mult)
                    for j in range(ln):
                        chunk_src[(stream, g0 + j)] = (mt, S8, j)
                return chunk_src

            for rep in range(reps):
                # -------- L1: full h1 table, 8-block groups --------
                for g0 in range(0, NBLK if with_compute else 0, 8):
                    nb = min(8, NBLK - g0)
                    qt = qmp.tile([2, 8 * P], F16, tag="qm")
                    nc.scalar.dma_start(
                        qt[:][:, :nb * P],
                        d_qm[:, g0 * P:(g0 + nb) * P])
                    h1x = h1p.tile([P, 8 * P], TDT, tag="h1x")
                    for h0 in range(0, nb, 4):
                        hn = min(4, nb - h0)
                        e4 = l1_ps.tile([P, 4 * P], F32, tag="e4")
                        for j in range(hn):
                            nc.tensor.matmul(
                                out=e4[:][:, j * P:(j + 1) * P],
                                lhsT=qt[:][:, (h0 + j) * P:(h0 + j + 1) * P],
                                rhs=W0b0[:], start=True, stop=True)
                        nc.scalar.activation(
                            out=h1x[:][:, h0 * P:(h0 + hn) * P],
                            in_=e4[:][:, :hn * P],
                            func=RELU, bias=0.0, scale=1.0)
                    nc.sync.dma_start(
                        table1[g0 * P:(g0 + nb) * P, :].rearrange(
                            "(a n) f -> n a f", n=P),
                        h1x[:][:, :nb * P].rearrange("p (a f) -> p a f", f=P))

                # -------- L2 --------
                chunk_src = emit_gathers(table1)
                for b in range(BLOCKS if with_compute else 0):
                    chunks = block_chunks(b)
                    aggT = agg_ps.tile([P, P], F32, tag="aggps")
                    for j, (stream, ci) in enumerate(chunks):
                        mt, S8, col = chunk_src[(stream, ci)]
                        nc.tensor.matmul(
                            out=aggT[:],
                            lhsT=mt[:][:, col * P:(col + 1) * P],
                            rhs=S8[:][:, col * P:(col + 1) * P],
                            start=(j == 0), stop=(j == len(chunks) - 1))
                    # z = W1^T @ aggT_h ; h2T = relu(z + b1) ; p2 = h2T^T@W2
                    aggs = hp.tile([P, P], F16, tag="aggs")
                    nc.vector.tensor_copy(aggs[:], aggT[:])
                    z_ps = p_ps.tile([P, P], F32, tag="pps")
                    nc.tensor.matmul(out=z_ps[:], lhsT=W1b[:], rhs=aggs[:],
                                     start=True, stop=True)
                    h2T = hp.tile([P, P], F16, tag="hT")
                    nc.scalar.activation(out=h2T[:], in_=z_ps[:],
                                         func=RELU, bias=b1c[:], scale=1.0)
                    p2_ps = agg_ps.tile([P, P], F32, tag="aggps")
                    nc.tensor.matmul(out=p2_ps[:], lhsT=h2T[:], rhs=W2b[:],
                                     start=True, stop=True)
                    p2_sb = hp.tile([P, P], TDT, tag="pout")
                    nc.vector.tensor_copy(p2_sb[:], p2_ps[:])
                    nc.sync.dma_start(slab2[b * P:(b + 1) * P, :], p2_sb[:])

                if with_coll:
                    nc.gpsimd.collective_compute(
                        "AllGather", mybir.AluOpType.bypass,
                        replica_groups=RG,
                        ins=[slab2[:]], outs=[table2[:]])

                # -------- L3 + readout --------
                chunk_src = emit_gathers(table2)
                rT = r_ps.tile([P, N_GRAPHS], F32, tag="rT",
                               name=f"rT_{rep}")
                for b in range(BLOCKS if with_compute else 0):
                    chunks = block_chunks(b)
                    agg = agg_ps.tile([P, P], F32, tag="aggps")
                    for j, (stream, ci) in enumerate(chunks):
                        mt, S8, col = chunk_src[(stream, ci)]
                        nc.tensor.matmul(
                            out=agg[:],
                            lhsT=S8[:][:, col * P:(col + 1) * P],
                            rhs=mt[:][:, col * P:(col + 1) * P],
                            start=(j == 0), stop=False)
                    # bias: agg += ones1^T @ b2row
                    nc.tensor.matmul(out=agg[:], lhsT=ones1[:], rhs=b2row[:],
                                     start=False, stop=True)
                    # h3 = relu(agg) * invc_dst  (scale>0 commutes w/ relu)
                    h3 = hp.tile([P, P], F16, tag="h3")
                    nc.scalar.activation(out=h3[:], in_=agg[:],
                                         func=RELU, bias=0.0,
                                         scale=invd[:][:, b:b + 1])
                    # readout: Sg [dst, 512] one-hot; rT += h3^T @ Sg
                    Sg = sp.tile([P, N_GRAPHS], F16, tag="Sg4")
                    nc.vector.tensor_scalar(
                        out=Sg[:], in0=iotg_f[:],
                        scalar1=gph[:][:, b:b + 1], scalar2=None,
                        op0=mybir.AluOpType.is_equal,
                        op1=mybir.AluOpType.bypass)
                    nc.tensor.matmul(
                        out=rT[:], lhsT=h3[:], rhs=Sg[:],
                        start=(b == 0), stop=(b == BLOCKS - 1))

                if with_compute:
                    r_sb = hp.tile([P, N_GRAPHS], F32, tag="rsb")
                    nc.vector.tensor_copy(r_sb[:], rT[:])
                    nc.sync.dma_start(partial[:, :], r_sb[:])

                if with_coll and with_compute:
                    nc.gpsimd.collective_compute(
                        "AllReduce", mybir.AluOpType.add, replica_groups=RG,
                        ins=[partial[:]], outs=[summed[:]])

                # -------- head: out[g,c] = rT_tile^T @ Wc + bc --------
                for t in range(NGT if with_compute else 0):
                    rg = hp.tile([P, P], F32, tag="hT")
                    nc.sync.dma_start(rg[:], summed[:, t * P:(t + 1) * P])
                    o_ps = p_ps.tile([P, N_CLASSES], F32, tag="pps")
                    nc.tensor.matmul(out=o_ps[:], lhsT=rg[:], rhs=Wc[:],
                                     start=True, stop=True)
                    o_sb = hp.tile([P, N_CLASSES], F32, tag="osb")
                    nc.vector.tensor_tensor(out=o_sb[:], in0=o_ps[:],
                                            in1=bcr[:],
                                            op=mybir.AluOpType.add)
                    nc.sync.dma_start(out[t * P:(t + 1) * P, :], o_sb[:])

    nc.compile()
    return nc


def make_in_maps(core_arrays, qm, W0, b0, W1, b1, W2, b2, Wc, bc):
    W0 = np.asarray(W0, np.float32).reshape(1, HID)
    b0 = np.asarray(b0, np.float32).reshape(1, HID)
    common = dict(
        qm=qm,
        W0b0=np.ascontiguousarray(
            np.concatenate([W0, b0], axis=0).astype(np.float16)),
        W1b=np.ascontiguousarray(np.asarray(W1, np.float32).astype(np.float16)),
        W2b=np.ascontiguousarray(np.asarray(W2, np.float32).astype(np.float16)),
        Wc=np.ascontiguousarray(Wc, np.float32),
        b1c=np.ascontiguousarray(b1, np.float32).reshape(P, 1),
        b2row=np.ascontiguousarray(
            np.asarray(b2, np.float32).reshape(1, HID).astype(np.float16)),
        bcr=np.ascontiguousarray(np.tile(
            np.asarray(bc, np.float32).reshape(1, N_CLASSES), (P, 1))),
    )
    in_maps = []
    for c in range(N_CORES):
        m = dict(common)
        ca = core_arrays[c]
        for k in ("idxA", "idxB", "dvA", "wA", "dvB", "wB", "gphv", "invd"):
            m[k] = ca[k]
        in_maps.append(m)
    return in_maps


_CACHE = {}


def _get_compiled(src, dst, graph_ids):
    import hashlib
    h = hashlib.md5()
    h.update(np.asarray(src).tobytes())
    h.update(np.asarray(dst).tobytes())
    h.update(np.asarray(graph_ids).tobytes())
    key = h.hexdigest()
    if key not in _CACHE:
        sched, core_arrays, qm = _prep_graph(src, dst, graph_ids)
        nc = build_nc(sched)
        _CACHE[key] = (nc, core_arrays, qm)
    return _CACHE[key]


def kernel(W0, b0, W1, b1, W2, b2, Wc, bc, src, dst, graph_ids,
           num_graphs=None, **_ignored):
    nc, core_arrays, qm = _get_compiled(src, dst, graph_ids)
    in_maps = make_in_maps(core_arrays, qm, W0, b0, W1, b1, W2, b2, Wc, bc)
    res = bass_utils.run_bass_kernel_spmd(
        nc, in_maps, core_ids=list(range(N_CORES)))
    o = res.results[0]["out"]
    return np.asarray(o, np.float32)
